# revision 2
# baseline (speedup 1.0000x reference)
"""Trainium2 Bass kernel for ExactVisionAttention — v3 (A2/B overlap).

Same math as v2 (3-term fp8e4 DoubleRow QKV/proj, bf16 QK/AV-swapped
attention, exp on ACT), restructured so the ACT exp stream starts ~70us
earlier: QKV head-group 0 computes first (wide, 8 psum banks), then
head-groups 1-2 run in a narrow 2-bank variant whose matmul groups are woven
between the first six heads' attention emission (B runs with a single s_ps
buffer during the overlap window), then heads 6-15 run at full rate.
"""

import os
import sys

for _p in ("/root/.axon_site/_ro/trn_rl_repo", "/opt/trn_rl_repo", "/root/.axon_site",
           "/root/.axon_site/_ro/pypackages"):
    if os.path.isdir(_p) and _p not in sys.path:
        sys.path.append(_p)

import numpy as np

S = 8192
HID = 1280
H = 16
D = 80
NSEG = 8
SEG = S // NSEG
MT = SEG // 128
KK = HID // 256
SCALE = float(D) ** -0.5

HS = 16.0
WS = 512.0
OS = 64.0
PS = 512.0

HGROUPS = [(0, 6), (6, 12), (12, 16)]

_CACHE = {}


def build_module(num_devices=8):
    import concourse.tile as tile
    from concourse import bacc, mybir
    from contextlib import ExitStack

    f32 = mybir.dt.float32
    bf16 = mybir.dt.bfloat16
    f8 = mybir.dt.float8e4
    Exp = mybir.ActivationFunctionType.Exp
    Copy = mybir.ActivationFunctionType.Copy
    DR = mybir.MatmulPerfMode.DoubleRow

    nc = bacc.Bacc("TRN2", target_bir_lowering=False, debug=False,
                   num_devices=num_devices)

    hidT_hi_in = nc.dram_tensor("hidT_hi", [KK, 128, 2 * SEG], f8,
                                kind="ExternalInput").ap()
    hidT_lo_in = nc.dram_tensor("hidT_lo", [KK, 128, 2 * SEG], f8,
                                kind="ExternalInput").ap()
    w_hi_in = nc.dram_tensor("w_hi", [KK, 128, 2, 3 * HID], f8,
                             kind="ExternalInput").ap()
    w_lo_in = nc.dram_tensor("w_lo", [KK, 128, 2, 3 * HID], f8,
                             kind="ExternalInput").ap()
    wp_hi_in = nc.dram_tensor("wp_hi", [KK, 128, 2 * HID], f8,
                              kind="ExternalInput").ap()
    wp_lo_in = nc.dram_tensor("wp_lo", [KK, 128, 2 * HID], f8,
                              kind="ExternalInput").ap()
    cos_in = nc.dram_tensor("cos40", [SEG, 40], f32, kind="ExternalInput").ap()
    sin_in = nc.dram_tensor("sin40", [SEG, 40], f32, kind="ExternalInput").ap()
    ident_in = nc.dram_tensor("identbf", [128, 128], bf16,
                              kind="ExternalInput").ap()
    out_dram = nc.dram_tensor("out", [SEG, HID], f32, kind="ExternalOutput").ap()

    with tile.TileContext(nc) as tc:
      with ExitStack() as ctx:
        constp = ctx.enter_context(tc.tile_pool(name="const", bufs=1))
        qkvp = ctx.enter_context(tc.tile_pool(name="qkvp", bufs=1))
        atp = ctx.enter_context(tc.tile_pool(name="atp", bufs=1))

        from concourse import library_config
        nc.gpsimd.load_library(library_config.proxy)

        # hidT first: the first QKV matmuls block on these (hi on SP, lo on
        # ACT so the two DGE queues split the transfer); weight DMAs follow
        # immediately on SP inside phase A.
        hidT_hi = [constp.tile([128, 2, SEG], f8, tag=f"hh{k2}", name=f"hh{k2}")
                   for k2 in range(KK)]
        hidT_lo = [constp.tile([128, 2, SEG], f8, tag=f"hl{k2}", name=f"hl{k2}")
                   for k2 in range(KK)]
        for k2 in range(KK):
            nc.sync.dma_start(hidT_hi[k2][:], hidT_hi_in[k2].rearrange(
                "p (two t) -> p two t", two=2))
            nc.scalar.dma_start(hidT_lo[k2][:], hidT_lo_in[k2].rearrange(
                "p (two t) -> p two t", two=2))

        cos40 = [constp.tile([128, 40], f32, tag=f"c{mt}", name=f"c{mt}")
                 for mt in range(MT)]
        sin40 = [constp.tile([128, 40], f32, tag=f"s{mt}", name=f"s{mt}")
                 for mt in range(MT)]
        for mt in range(MT):
            nc.scalar.dma_start(cos40[mt][:], cos_in[mt * 128:(mt + 1) * 128, :])
            nc.scalar.dma_start(sin40[mt][:], sin_in[mt * 128:(mt + 1) * 128, :])
        ident = constp.tile([128, 128], bf16, tag="ident", name="ident")
        nc.scalar.dma_start(ident[:], ident_in[:])

        # q/k/v split per head-group so overlapped A2 writes (groups 1-2)
        # never false-depend against B reads of group-0 heads
        NHG = [hg[1] - hg[0] for hg in HGROUPS]
        q_sb = [[qkvp.tile([128, NHG[g], D], bf16, tag=f"q{g}_{mt}",
                           name=f"q{g}_{mt}") for mt in range(MT)]
                for g in range(3)]
        k_sb = [[qkvp.tile([128, NHG[g], D], bf16, tag=f"k{g}_{mt}",
                           name=f"k{g}_{mt}") for mt in range(MT)]
                for g in range(3)]
        v_sb = [[qkvp.tile([128, NHG[g], D + 1], bf16, tag=f"v{g}_{mt}",
                           name=f"v{g}_{mt}") for mt in range(MT)]
                for g in range(3)]
        for g in range(3):
            for mt in range(MT):
                nc.gpsimd.memset(v_sb[g][mt][:, :, D:D + 1], 1.0 / OS)

        aT_hi = [atp.tile([128, 2, SEG], f8, tag=f"ah{k2}", name=f"ah{k2}")
                 for k2 in range(KK)]
        aT_lo = [atp.tile([128, 2, SEG], f8, tag=f"al{k2}", name=f"al{k2}")
                 for k2 in range(KK)]
        wp_hi = [atp.tile([128, 2, HID], f8, tag=f"wph{k2}", name=f"wph{k2}")
                 for k2 in range(KK)]
        wp_lo = [atp.tile([128, 2, HID], f8, tag=f"wpl{k2}", name=f"wpl{k2}")
                 for k2 in range(KK)]
        for k2 in range(KK):
            nc.gpsimd.dma_start(wp_hi[k2][:], wp_hi_in[k2].rearrange(
                "p (two e) -> p two e", two=2))
            nc.gpsimd.dma_start(wp_lo[k2][:], wp_lo_in[k2].rearrange(
                "p (two e) -> p two e", two=2))

        # ----- shared emitters -------------------------------------------

        def rope_evict(g, grp, mt, ps, nh, h0g, rtp, stage_on_act):
            """ps: [128, nh*D] psum (or staged sbuf) view for this mt."""
            if grp == 2:
                nc.vector.tensor_scalar_mul(
                    v_sb[g][mt][:, :, 0:D],
                    ps.rearrange("p (h d) -> p h d", h=nh), 1.0 / (HS * WS))
                return
            if stage_on_act:
                qs = rtp.tile([128, 512], f32, tag="qs", bufs=4, name="qs")
                nc.scalar.copy(qs[:, 0:nh * D], ps)
                ps = qs[:, 0:nh * D]
            dst = q_sb[g][mt] if grp == 0 else k_sb[g][mt]
            ps3 = ps.rearrange("p (h d) -> p h d", h=nh)
            ps4 = ps.rearrange("p (h two d) -> p h two d", h=nh, two=2)
            cos_bc4 = (cos40[mt][:].unsqueeze(1).unsqueeze(2)
                       .broadcast_to([128, nh, 2, 40]))
            sin_bc3 = (sin40[mt][:].unsqueeze(1).broadcast_to([128, nh, 40]))
            t = rtp.tile([128, 6, D], f32, tag="t", name="t")
            t4 = t[:, 0:nh, :].rearrange("p h (two d) -> p h two d", two=2)
            nc.vector.tensor_mul(t4, ps4, cos_bc4)
            m1 = rtp.tile([128, 6, 40], f32, tag="m1", name="m1")
            nc.vector.tensor_mul(m1[:, 0:nh, :], ps3[:, :, 40:80], sin_bc3)
            m2 = rtp.tile([128, 6, 40], f32, tag="m2", name="m2")
            nc.vector.tensor_mul(m2[:, 0:nh, :], ps3[:, :, 0:40], sin_bc3)
            nc.gpsimd.tensor_sub(dst[:, :, 0:40], t[:, 0:nh, 0:40],
                                 m1[:, 0:nh, :])
            nc.gpsimd.tensor_add(dst[:, :, 40:80], m2[:, 0:nh, :],
                                 t[:, 0:nh, 40:80])

        def chunk_wide(g, grp, wp, rtp, psA):
            """8-bank QKV chunk: all 8 mt tiles accumulate concurrently."""
            h0, h1 = HGROUPS[g]
            nh = h1 - h0
            cw = nh * D
            c0 = (h0 * 3 + grp * nh) * D
            pss = [psA.tile([128, 512], f32, tag=f"pa{mt}", name=f"pa{mt}")
                   for mt in range(MT)]
            nn_splits = [(0, cw // 2), (cw // 2, cw // 2)]
            for k2 in range(KK):
                wt_hi = wp.tile([128, 2, 512], f8, tag="wth", name="wth",
                                bufs=10)
                nc.sync.dma_start(wt_hi[:, :, 0:cw],
                                  w_hi_in[k2, :, :, c0:c0 + cw])
                wt_lo = wp.tile([128, 2, 512], f8, tag="wtl", name="wtl",
                                bufs=10)
                nc.sync.dma_start(wt_lo[:, :, 0:cw],
                                  w_lo_in[k2, :, :, c0:c0 + cw])
                for mt in range(MT):
                    lhs_hi = hidT_hi[k2][:, :, mt * 128:(mt + 1) * 128]
                    lhs_lo = hidT_lo[k2][:, :, mt * 128:(mt + 1) * 128]
                    for si, (n0, nw) in enumerate(nn_splits):
                        dst = pss[mt][:, n0:n0 + nw]
                        first = (k2 == 0) and (si == 0)
                        last = (k2 == KK - 1) and (si == len(nn_splits) - 1)
                        nc.tensor.matmul(dst, lhs_hi, wt_hi[:, :, n0:n0 + nw],
                                         start=first, stop=False, perf_mode=DR)
                        nc.tensor.matmul(dst, lhs_hi, wt_lo[:, :, n0:n0 + nw],
                                         start=False, stop=False, perf_mode=DR)
                        nc.tensor.matmul(dst, lhs_lo, wt_hi[:, :, n0:n0 + nw],
                                         start=False, stop=last, perf_mode=DR)
            for mt in range(MT):
                rope_evict(g, grp, mt, pss[mt][:, 0:cw], nh, h0, rtp,
                           stage_on_act=True)

        def chunk_narrow_gen(g, grp, wp, rtp, psA):
            """2-bank QKV chunk as a generator: yields after each (pass, k2)
            matmul group so B-head emission can weave between them."""
            h0, h1 = HGROUPS[g]
            nh = h1 - h0
            cw = nh * D
            c0 = (h0 * 3 + grp * nh) * D
            wts = []
            for k2 in range(KK):
                wt_hi = wp.tile([128, 2, 512], f8, tag="wth", name="wth",
                                bufs=10)
                nc.sync.dma_start(wt_hi[:, :, 0:cw],
                                  w_hi_in[k2, :, :, c0:c0 + cw])
                wt_lo = wp.tile([128, 2, 512], f8, tag="wtl", name="wtl",
                                bufs=10)
                nc.sync.dma_start(wt_lo[:, :, 0:cw],
                                  w_lo_in[k2, :, :, c0:c0 + cw])
                wts.append((wt_hi, wt_lo))
            nn_splits = [(0, cw // 2), (cw // 2, cw // 2)]
            for mt in range(MT):
                ps = psA.tile([128, 512], f32, tag="pan", name="pan", bufs=2)
                for k2 in range(KK):
                    wt_hi, wt_lo = wts[k2]
                    lhs_hi = hidT_hi[k2][:, :, mt * 128:(mt + 1) * 128]
                    lhs_lo = hidT_lo[k2][:, :, mt * 128:(mt + 1) * 128]
                    for si, (n0, nw) in enumerate(nn_splits):
                        dst = ps[:, n0:n0 + nw]
                        first = (k2 == 0) and (si == 0)
                        last = (k2 == KK - 1) and (si == 1)
                        nc.tensor.matmul(dst, lhs_hi, wt_hi[:, :, n0:n0 + nw],
                                         start=first, stop=False, perf_mode=DR)
                        nc.tensor.matmul(dst, lhs_hi, wt_lo[:, :, n0:n0 + nw],
                                         start=False, stop=False, perf_mode=DR)
                        nc.tensor.matmul(dst, lhs_lo, wt_hi[:, :, n0:n0 + nw],
                                         start=False, stop=last, perf_mode=DR)
                    yield
                # RoPE reads psum directly (ACT stays exp-only here)
                rope_evict(g, grp, mt, ps[:, 0:cw], nh, h0, rtp,
                           stage_on_act=False)
                yield

        def make_head_emitter(sbB, psB, s_bufs):
            state = {"backlog": None, "avcopy": None, "tail": None,
                     "tail_ao": None}

            def qkv_of(h):
                g = 0 if h < 6 else (1 if h < 12 else 2)
                return g, h - HGROUPS[g][0]

            def emit_head(h, weave=None):
                def W():
                    if weave is not None:
                        weave()
                g, hc = qkv_of(h)
                qkT_sb = sbB.tile([D, 2 * SEG], bf16, tag="qkT", bufs=2,
                                  name="qkT")
                tp = psB.tile([D, SEG], bf16, tag="s", bufs=2, name="tpq")
                for mt in range(MT):
                    nc.tensor.transpose(tp[:, mt * 128:(mt + 1) * 128],
                                        q_sb[g][mt][:, hc, :], ident[:])
                nc.vector.tensor_copy(qkT_sb[:, 0:SEG], tp[:])
                W()
                tp2 = psB.tile([D, SEG], bf16, tag="s", bufs=2, name="tpk")
                for mt in range(MT):
                    nc.tensor.transpose(tp2[:, mt * 128:(mt + 1) * 128],
                                        k_sb[g][mt][:, hc, :], ident[:])
                nc.vector.tensor_copy(qkT_sb[:, SEG:2 * SEG], tp2[:])
                W()

                av_ps = [psB.tile([128, MT // 2, D + 1], f32, tag=f"av{i}",
                                  bufs=1, name=f"av{i}") for i in range(2)]
                p_tiles = [None] * MT

                def emit_qk(kc):
                    s_ps = psB.tile([128, SEG], f32, tag="s", bufs=s_bufs,
                                    name="s")
                    for nn in range(2):
                        nc.tensor.matmul(
                            s_ps[:, nn * 512:(nn + 1) * 512],
                            qkT_sb[:, SEG + kc * 128:SEG + (kc + 1) * 128],
                            qkT_sb[:, nn * 512:(nn + 1) * 512],
                            start=True, stop=True)
                    p_sb = sbB.tile([128, SEG], bf16, tag="p", bufs=5,
                                    name="p")
                    nc.scalar.activation(p_sb[:], s_ps[:], Exp, scale=SCALE)
                    p_tiles[kc] = p_sb

                def emit_av(kc, g=g, hc=hc):
                    for qt in range(MT):
                        half, qi = divmod(qt, MT // 2)
                        nc.tensor.matmul(
                            av_ps[half][:, qi, :],
                            p_tiles[kc][:, qt * 128:(qt + 1) * 128],
                            v_sb[g][kc][:, hc, :],
                            start=(kc == 0 and qi == 0),
                            stop=(kc == MT - 1 and qi == MT // 2 - 1))

                def emit_avcopy(av_ps=av_ps):
                    # evict psum->SBUF right away so av_ps recycles for the
                    # next head without waiting on the rest of the tail
                    ao = sbB.tile([128, MT, D + 1], f32, tag="ao", bufs=2,
                                  name="ao")
                    for i in range(2):
                        nc.vector.tensor_copy(
                            ao[:, i * (MT // 2):(i + 1) * (MT // 2), :],
                            av_ps[i][:])
                    return ao

                def emit_tail(ao, h=h):
                    an = sbB.tile([128, MT, D], bf16, tag="an", name="an")
                    rcp = sbB.tile([128, MT], f32, tag="rcp", name="rcp")
                    nc.vector.reciprocal(rcp[:], ao[:, :, D])
                    # all-SBUF multiply: runs on Pool (DVE is the busy one)
                    nc.gpsimd.tensor_mul(
                        an[:], ao[:, :, 0:D],
                        rcp[:].unsqueeze(2).broadcast_to([128, MT, D]))
                    aT_ps = psB.tile([D, SEG], bf16, tag="pan", bufs=2,
                                     name="aTps")
                    for qt in range(MT):
                        nc.tensor.transpose(aT_ps[:, qt * 128:(qt + 1) * 128],
                                            an[:, qt, :], ident[:])
                    hi8 = sbB.tile([D, SEG], f8, tag="hi8", name="hi8")
                    nc.vector.tensor_copy(hi8[:], aT_ps[:])
                    lo8 = sbB.tile([D, SEG], f8, tag="lo8", name="lo8")
                    nc.vector.tensor_sub(lo8[:], aT_ps[:], hi8[:])
                    r0 = h * D
                    j = 0
                    while j < D:
                        r = r0 + j
                        k2, sl, p = r // 256, (r // 128) % 2, r % 128
                        ln = min(D - j, 128 - p)
                        nc.sync.dma_start(aT_hi[k2][p:p + ln, sl, :],
                                          hi8[j:j + ln, :])
                        nc.sync.dma_start(aT_lo[k2][p:p + ln, sl, :],
                                          lo8[j:j + ln, :])
                        j += ln

                emit_qk(0)
                emit_qk(1)
                W()
                if state["backlog"] is not None:
                    state["backlog"](6)
                emit_qk(2)
                if state["backlog"] is not None:
                    state["backlog"](7)
                    state["backlog"] = None
                if state["avcopy"] is not None:
                    state["tail_ao"] = state["avcopy"]()
                    state["avcopy"] = None
                W()
                emit_av(0)
                for kc in range(3, MT):
                    emit_qk(kc)
                    W()
                    emit_av(kc - 2)
                    if kc == 5 and state["tail"] is not None:
                        # deferred tail of the previous head: placed here so
                        # its PE transposes never delay this head's QKs
                        state["tail"](state["tail_ao"])
                        state["tail"] = None
                state["backlog"] = emit_av
                state["avcopy"] = emit_avcopy
                state["tail"] = emit_tail

            def drain():
                if state["backlog"] is not None:
                    state["backlog"](6)
                    state["backlog"](7)
                    state["backlog"] = None
                if state["avcopy"] is not None:
                    state["tail_ao"] = state["avcopy"]()
                    state["avcopy"] = None
                if state["tail"] is not None:
                    state["tail"](state["tail_ao"])
                    state["tail"] = None

            return emit_head, drain

        # ----- Scope 1: head-group 0 QKV (wide, ACT idle) -----------------
        with ExitStack() as s1:
            wp1 = s1.enter_context(tc.tile_pool(name="wp1", bufs=10))
            rtp1 = s1.enter_context(tc.tile_pool(name="rtp1", bufs=2))
            psA1 = s1.enter_context(tc.tile_pool(name="psA1", bufs=1,
                                                 space="PSUM"))
            for grp in range(3):
                chunk_wide(0, grp, wp1, rtp1, psA1)

        # ----- Scope 2: all heads, narrow QKV groups 1-2 woven in ---------
        with ExitStack() as s2:
            wp2 = s2.enter_context(tc.tile_pool(name="wp2", bufs=10))
            rtp2 = s2.enter_context(tc.tile_pool(name="rtp2", bufs=2))
            sbB1 = s2.enter_context(tc.tile_pool(name="sbB1", bufs=2))
            psB1 = s2.enter_context(tc.tile_pool(name="psB1", bufs=1,
                                                 space="PSUM"))

            progress = {"g": 1}

            def a2_rest():
                for g in (1, 2):
                    for grp in range(3):
                        yield from chunk_narrow_gen(g, grp, wp2, rtp2, psB1)
                    progress["g"] = g + 1
            gen = a2_rest()

            def weave(gen=gen):
                next(gen, None)
                next(gen, None)

            emit_head, drain = make_head_emitter(sbB1, psB1, s_bufs=2)
            for h in range(H):
                # a head's q/k/v must be fully emitted before the head reads it
                need = 1 if h < 6 else (2 if h < 12 else 3)
                while progress["g"] < need and next(gen, "end") != "end":
                    pass
                emit_head(h, weave=weave)
            drain()
            for _ in gen:
                pass

        # ----- Phase C: projection (3-term fp8 DR) ------------------------
        with ExitStack() as cctx:
            osb = cctx.enter_context(tc.tile_pool(name="osb", bufs=1))
            psC = cctx.enter_context(tc.tile_pool(name="psC", bufs=1,
                                                  space="PSUM"))
            NW = [(i * 256, 256) for i in range(5)]
            for mt in range(MT):
                pc = psC.tile([128, HID], f32, tag="pc", bufs=2, name="pc")
                for k2 in range(KK):
                    lh_hi = aT_hi[k2][:, :, mt * 128:(mt + 1) * 128]
                    lh_lo = aT_lo[k2][:, :, mt * 128:(mt + 1) * 128]
                    for (n0, nw) in NW:
                        dst = pc[:, n0:n0 + nw]
                        first = (k2 == 0) and (n0 % 512 == 0)
                        last = (k2 == KK - 1) and n0 in (256, 768, 1024)
                        nc.tensor.matmul(dst, lh_hi,
                                         wp_hi[k2][:, :, n0:n0 + nw],
                                         start=first, stop=False, perf_mode=DR)
                        nc.tensor.matmul(dst, lh_hi,
                                         wp_lo[k2][:, :, n0:n0 + nw],
                                         start=False, stop=False, perf_mode=DR)
                        nc.tensor.matmul(dst, lh_lo,
                                         wp_hi[k2][:, :, n0:n0 + nw],
                                         start=False, stop=last, perf_mode=DR)
                ot = osb.tile([128, HID], f32, tag="ot", bufs=2, name="ot")
                nc.scalar.activation(ot[:, 0:640], pc[:, 0:640], Copy,
                                     scale=1.0 / (OS * PS))
                nc.vector.tensor_scalar_mul(ot[:, 640:HID], pc[:, 640:HID],
                                            1.0 / (OS * PS))
                nc.sync.dma_start(out_dram[mt * 128:(mt + 1) * 128, :], ot[:])

    nc.compile()
    return nc


# ----- host-side prep (shared with v2) --------------------------------------

def _f8(x):
    import ml_dtypes
    return np.clip(x, -440.0, 440.0).astype(ml_dtypes.float8_e4m3)


def _hi_lo(x, scale):
    xs = np.asarray(x, np.float32) * scale
    hi = _f8(xs)
    lo = _f8(xs - hi.astype(np.float32))
    return hi, lo


def _col_perm():
    perm = []
    for (h0, h1) in HGROUPS:
        for grp in range(3):
            for h in range(h0, h1):
                base = grp * H * D + h * D
                perm.extend(range(base, base + D))
    return np.asarray(perm)


def _pack_pairs(a):
    n = a.shape[1]
    return np.ascontiguousarray(a.reshape(KK, 2, 128, n).transpose(0, 2, 1, 3))


def prepare_in_maps(hidden_states, cos, sin, wqkv, wproj):
    import ml_dtypes

    wq = wqkv[:, _col_perm()]
    w_hi, w_lo = _hi_lo(wq, WS)
    w_hi = _pack_pairs(w_hi).reshape(KK, 128, 2, 3 * HID)
    w_lo = _pack_pairs(w_lo).reshape(KK, 128, 2, 3 * HID)
    wp_hi, wp_lo = _hi_lo(wproj, PS)
    wp_hi = _pack_pairs(wp_hi).reshape(KK, 128, 2 * HID)
    wp_lo = _pack_pairs(wp_lo).reshape(KK, 128, 2 * HID)
    cos40 = np.ascontiguousarray(cos[:, 0:40]) / np.float32(HS * WS)
    sin40 = np.ascontiguousarray(sin[:, 0:40]) / np.float32(HS * WS)
    identbf = np.eye(128, dtype=ml_dtypes.bfloat16)

    in_maps = []
    for c in range(NSEG):
        rows = slice(c * SEG, (c + 1) * SEG)
        hT_hi, hT_lo = _hi_lo(hidden_states[rows].T, HS)
        in_maps.append({
            "hidT_hi": _pack_pairs(hT_hi).reshape(KK, 128, 2 * SEG),
            "hidT_lo": _pack_pairs(hT_lo).reshape(KK, 128, 2 * SEG),
            "w_hi": w_hi, "w_lo": w_lo,
            "wp_hi": wp_hi, "wp_lo": wp_lo,
            "cos40": np.ascontiguousarray(cos40[rows]),
            "sin40": np.ascontiguousarray(sin40[rows]),
            "identbf": identbf,
        })
    return in_maps


def _get_module():
    if "nc" not in _CACHE:
        _CACHE["nc"] = build_module(num_devices=NSEG)
    return _CACHE["nc"]


def kernel(hidden_states, cos, sin, qkv_kernel, qkv_bias, proj_kernel,
           proj_bias, cu_seqlens):
    from concourse import bass_utils

    hidden_states = np.ascontiguousarray(hidden_states, dtype=np.float32)
    cos = np.ascontiguousarray(cos, dtype=np.float32)
    sin = np.ascontiguousarray(sin, dtype=np.float32)
    wqkv = np.asarray(qkv_kernel, dtype=np.float32).reshape(HID, 3 * H * D)
    wproj = np.ascontiguousarray(proj_kernel, dtype=np.float32)

    assert not np.any(np.asarray(qkv_bias)), "nonzero qkv_bias unsupported"
    assert not np.any(np.asarray(proj_bias)), "nonzero proj_bias unsupported"
    expected_cu = np.arange(NSEG + 1, dtype=np.int64) * SEG
    assert np.array_equal(np.asarray(cu_seqlens, dtype=np.int64), expected_cu), \
        "kernel specialized for equal 1024-token segments"

    in_maps = prepare_in_maps(hidden_states, cos, sin, wqkv, wproj)
    nc = _get_module()
    res = bass_utils.run_bass_kernel_spmd(nc, in_maps,
                                          core_ids=list(range(NSEG)))
    out = np.concatenate([res.results[c]["out"] for c in range(NSEG)], axis=0)
    return out.astype(np.float32)


# revision 3
# speedup vs baseline: 1.0115x; 1.0115x over previous
"""Trainium2 Bass kernel for ExactVisionAttention — v3 (A2/B overlap).

Same math as v2 (3-term fp8e4 DoubleRow QKV/proj, bf16 QK/AV-swapped
attention, exp on ACT), restructured so the ACT exp stream starts ~70us
earlier: QKV head-group 0 computes first (wide, 8 psum banks), then
head-groups 1-2 run in a narrow 2-bank variant whose matmul groups are woven
between the first six heads' attention emission (B runs with a single s_ps
buffer during the overlap window), then heads 6-15 run at full rate.
"""

import os
import sys

for _p in ("/root/.axon_site/_ro/trn_rl_repo", "/opt/trn_rl_repo", "/root/.axon_site",
           "/root/.axon_site/_ro/pypackages"):
    if os.path.isdir(_p) and _p not in sys.path:
        sys.path.append(_p)

import numpy as np

S = 8192
HID = 1280
H = 16
D = 80
NSEG = 8
SEG = S // NSEG
MT = SEG // 128
KK = HID // 256
SCALE = float(D) ** -0.5

HS = 16.0
WS = 512.0
OS = 64.0
PS = 512.0

HGROUPS = [(0, 6), (6, 12), (12, 16)]

_CACHE = {}


def build_module(num_devices=8):
    import concourse.tile as tile
    from concourse import bacc, mybir
    from contextlib import ExitStack

    f32 = mybir.dt.float32
    bf16 = mybir.dt.bfloat16
    f8 = mybir.dt.float8e4
    Exp = mybir.ActivationFunctionType.Exp
    Copy = mybir.ActivationFunctionType.Copy
    DR = mybir.MatmulPerfMode.DoubleRow

    nc = bacc.Bacc("TRN2", target_bir_lowering=False, debug=False,
                   num_devices=num_devices)

    hidT_hi_in = nc.dram_tensor("hidT_hi", [KK, 128, 2 * SEG], f8,
                                kind="ExternalInput").ap()
    hidT_lo_in = nc.dram_tensor("hidT_lo", [KK, 128, 2 * SEG], f8,
                                kind="ExternalInput").ap()
    w_hi_in = nc.dram_tensor("w_hi", [KK, 128, 2, 3 * HID], f8,
                             kind="ExternalInput").ap()
    w_lo_in = nc.dram_tensor("w_lo", [KK, 128, 2, 3 * HID], f8,
                             kind="ExternalInput").ap()
    wp_hi_in = nc.dram_tensor("wp_hi", [KK, 128, 2 * HID], f8,
                              kind="ExternalInput").ap()
    wp_lo_in = nc.dram_tensor("wp_lo", [KK, 128, 2 * HID], f8,
                              kind="ExternalInput").ap()
    cos_in = nc.dram_tensor("cos40", [SEG, 40], f32, kind="ExternalInput").ap()
    sin_in = nc.dram_tensor("sin40", [SEG, 40], f32, kind="ExternalInput").ap()
    ident_in = nc.dram_tensor("identbf", [128, 128], bf16,
                              kind="ExternalInput").ap()
    out_dram = nc.dram_tensor("out", [SEG, HID], f32, kind="ExternalOutput").ap()

    with tile.TileContext(nc) as tc:
      with ExitStack() as ctx:
        constp = ctx.enter_context(tc.tile_pool(name="const", bufs=1))
        qkvp = ctx.enter_context(tc.tile_pool(name="qkvp", bufs=1))
        atp = ctx.enter_context(tc.tile_pool(name="atp", bufs=1))

        from concourse import library_config
        nc.gpsimd.load_library(library_config.proxy)

        # hidT first: the first QKV matmuls block on these (hi on SP, lo on
        # ACT so the two DGE queues split the transfer); weight DMAs follow
        # immediately on SP inside phase A.
        hidT_hi = [constp.tile([128, 2, SEG], f8, tag=f"hh{k2}", name=f"hh{k2}")
                   for k2 in range(KK)]
        hidT_lo = [constp.tile([128, 2, SEG], f8, tag=f"hl{k2}", name=f"hl{k2}")
                   for k2 in range(KK)]
        for k2 in range(KK):
            nc.sync.dma_start(hidT_hi[k2][:], hidT_hi_in[k2].rearrange(
                "p (two t) -> p two t", two=2))
            nc.scalar.dma_start(hidT_lo[k2][:], hidT_lo_in[k2].rearrange(
                "p (two t) -> p two t", two=2))

        cos40 = [constp.tile([128, 40], f32, tag=f"c{mt}", name=f"c{mt}")
                 for mt in range(MT)]
        sin40 = [constp.tile([128, 40], f32, tag=f"s{mt}", name=f"s{mt}")
                 for mt in range(MT)]
        for mt in range(MT):
            nc.scalar.dma_start(cos40[mt][:], cos_in[mt * 128:(mt + 1) * 128, :])
            nc.scalar.dma_start(sin40[mt][:], sin_in[mt * 128:(mt + 1) * 128, :])
        ident = constp.tile([128, 128], bf16, tag="ident", name="ident")
        nc.scalar.dma_start(ident[:], ident_in[:])

        # q/k/v split per head-group so overlapped A2 writes (groups 1-2)
        # never false-depend against B reads of group-0 heads
        NHG = [hg[1] - hg[0] for hg in HGROUPS]
        q_sb = [[qkvp.tile([128, NHG[g], D], bf16, tag=f"q{g}_{mt}",
                           name=f"q{g}_{mt}") for mt in range(MT)]
                for g in range(3)]
        k_sb = [[qkvp.tile([128, NHG[g], D], bf16, tag=f"k{g}_{mt}",
                           name=f"k{g}_{mt}") for mt in range(MT)]
                for g in range(3)]
        v_sb = [[qkvp.tile([128, NHG[g], D + 1], bf16, tag=f"v{g}_{mt}",
                           name=f"v{g}_{mt}") for mt in range(MT)]
                for g in range(3)]
        for g in range(3):
            for mt in range(MT):
                nc.gpsimd.memset(v_sb[g][mt][:, :, D:D + 1], 1.0 / OS)

        aT_hi = [atp.tile([128, 2, SEG], f8, tag=f"ah{k2}", name=f"ah{k2}")
                 for k2 in range(KK)]
        aT_lo = [atp.tile([128, 2, SEG], f8, tag=f"al{k2}", name=f"al{k2}")
                 for k2 in range(KK)]
        wp_hi = [atp.tile([128, 2, HID], f8, tag=f"wph{k2}", name=f"wph{k2}")
                 for k2 in range(KK)]
        wp_lo = [atp.tile([128, 2, HID], f8, tag=f"wpl{k2}", name=f"wpl{k2}")
                 for k2 in range(KK)]
        for k2 in range(KK):
            nc.gpsimd.dma_start(wp_hi[k2][:], wp_hi_in[k2].rearrange(
                "p (two e) -> p two e", two=2))
            nc.gpsimd.dma_start(wp_lo[k2][:], wp_lo_in[k2].rearrange(
                "p (two e) -> p two e", two=2))

        # ----- shared emitters -------------------------------------------

        def rope_evict(g, grp, mt, ps, nh, h0g, rtp, stage_on_act):
            """ps: [128, nh*D] psum (or staged sbuf) view for this mt."""
            if grp == 2:
                nc.vector.tensor_scalar_mul(
                    v_sb[g][mt][:, :, 0:D],
                    ps.rearrange("p (h d) -> p h d", h=nh), 1.0 / (HS * WS))
                return
            if stage_on_act:
                qs = rtp.tile([128, 512], f32, tag="qs", bufs=4, name="qs")
                nc.scalar.copy(qs[:, 0:nh * D], ps)
                ps = qs[:, 0:nh * D]
            dst = q_sb[g][mt] if grp == 0 else k_sb[g][mt]
            ps3 = ps.rearrange("p (h d) -> p h d", h=nh)
            ps4 = ps.rearrange("p (h two d) -> p h two d", h=nh, two=2)
            cos_bc4 = (cos40[mt][:].unsqueeze(1).unsqueeze(2)
                       .broadcast_to([128, nh, 2, 40]))
            sin_bc3 = (sin40[mt][:].unsqueeze(1).broadcast_to([128, nh, 40]))
            t = rtp.tile([128, 6, D], f32, tag="t", name="t")
            t4 = t[:, 0:nh, :].rearrange("p h (two d) -> p h two d", two=2)
            nc.vector.tensor_mul(t4, ps4, cos_bc4)
            m1 = rtp.tile([128, 6, 40], f32, tag="m1", name="m1")
            nc.vector.tensor_mul(m1[:, 0:nh, :], ps3[:, :, 40:80], sin_bc3)
            m2 = rtp.tile([128, 6, 40], f32, tag="m2", name="m2")
            nc.vector.tensor_mul(m2[:, 0:nh, :], ps3[:, :, 0:40], sin_bc3)
            nc.gpsimd.tensor_sub(dst[:, :, 0:40], t[:, 0:nh, 0:40],
                                 m1[:, 0:nh, :])
            nc.gpsimd.tensor_add(dst[:, :, 40:80], m2[:, 0:nh, :],
                                 t[:, 0:nh, 40:80])

        def chunk_wide(g, grp, wp, rtp, psA):
            """8-bank QKV chunk: all 8 mt tiles accumulate concurrently."""
            h0, h1 = HGROUPS[g]
            nh = h1 - h0
            cw = nh * D
            c0 = (h0 * 3 + grp * nh) * D
            pss = [psA.tile([128, 512], f32, tag=f"pa{mt}", name=f"pa{mt}")
                   for mt in range(MT)]
            nn_splits = [(0, cw // 2), (cw // 2, cw // 2)]
            for k2 in range(KK):
                wt_hi = wp.tile([128, 2, 512], f8, tag="wth", name="wth",
                                bufs=10)
                nc.sync.dma_start(wt_hi[:, :, 0:cw],
                                  w_hi_in[k2, :, :, c0:c0 + cw])
                wt_lo = wp.tile([128, 2, 512], f8, tag="wtl", name="wtl",
                                bufs=10)
                nc.sync.dma_start(wt_lo[:, :, 0:cw],
                                  w_lo_in[k2, :, :, c0:c0 + cw])
                for mt in range(MT):
                    lhs_hi = hidT_hi[k2][:, :, mt * 128:(mt + 1) * 128]
                    lhs_lo = hidT_lo[k2][:, :, mt * 128:(mt + 1) * 128]
                    for si, (n0, nw) in enumerate(nn_splits):
                        dst = pss[mt][:, n0:n0 + nw]
                        first = (k2 == 0) and (si == 0)
                        last = (k2 == KK - 1) and (si == len(nn_splits) - 1)
                        nc.tensor.matmul(dst, lhs_hi, wt_hi[:, :, n0:n0 + nw],
                                         start=first, stop=False, perf_mode=DR)
                        nc.tensor.matmul(dst, lhs_hi, wt_lo[:, :, n0:n0 + nw],
                                         start=False, stop=False, perf_mode=DR)
                        nc.tensor.matmul(dst, lhs_lo, wt_hi[:, :, n0:n0 + nw],
                                         start=False, stop=last, perf_mode=DR)
            for mt in range(MT):
                rope_evict(g, grp, mt, pss[mt][:, 0:cw], nh, h0, rtp,
                           stage_on_act=True)

        def chunk_narrow_gen(g, grp, wp, rtp, psA):
            """2-bank QKV chunk as a generator: yields after each (pass, k2)
            matmul group so B-head emission can weave between them."""
            h0, h1 = HGROUPS[g]
            nh = h1 - h0
            cw = nh * D
            c0 = (h0 * 3 + grp * nh) * D
            wts = []
            for k2 in range(KK):
                wt_hi = wp.tile([128, 2, 512], f8, tag="wth", name="wth",
                                bufs=10)
                nc.sync.dma_start(wt_hi[:, :, 0:cw],
                                  w_hi_in[k2, :, :, c0:c0 + cw])
                wt_lo = wp.tile([128, 2, 512], f8, tag="wtl", name="wtl",
                                bufs=10)
                nc.sync.dma_start(wt_lo[:, :, 0:cw],
                                  w_lo_in[k2, :, :, c0:c0 + cw])
                wts.append((wt_hi, wt_lo))
            nn_splits = [(0, cw // 2), (cw // 2, cw // 2)]
            for mt in range(MT):
                ps = psA.tile([128, 512], f32, tag="pan", name="pan", bufs=2)
                for k2 in range(KK):
                    wt_hi, wt_lo = wts[k2]
                    lhs_hi = hidT_hi[k2][:, :, mt * 128:(mt + 1) * 128]
                    lhs_lo = hidT_lo[k2][:, :, mt * 128:(mt + 1) * 128]
                    for si, (n0, nw) in enumerate(nn_splits):
                        dst = ps[:, n0:n0 + nw]
                        first = (k2 == 0) and (si == 0)
                        last = (k2 == KK - 1) and (si == 1)
                        nc.tensor.matmul(dst, lhs_hi, wt_hi[:, :, n0:n0 + nw],
                                         start=first, stop=False, perf_mode=DR)
                        nc.tensor.matmul(dst, lhs_hi, wt_lo[:, :, n0:n0 + nw],
                                         start=False, stop=False, perf_mode=DR)
                        nc.tensor.matmul(dst, lhs_lo, wt_hi[:, :, n0:n0 + nw],
                                         start=False, stop=last, perf_mode=DR)
                    yield
                # RoPE reads psum directly (ACT stays exp-only here)
                rope_evict(g, grp, mt, ps[:, 0:cw], nh, h0, rtp,
                           stage_on_act=False)
                yield

        def make_head_emitter(sbB, psB, s_bufs):
            state = {"backlog": None, "avcopy": None, "tail": None,
                     "tail_ao": None}

            def qkv_of(h):
                g = 0 if h < 6 else (1 if h < 12 else 2)
                return g, h - HGROUPS[g][0]

            def emit_head(h, weave=None):
                def W():
                    if weave is not None:
                        weave()
                g, hc = qkv_of(h)
                qkT_sb = sbB.tile([D, 2 * SEG], bf16, tag="qkT", bufs=2,
                                  name="qkT")
                tp = psB.tile([D, SEG], bf16, tag="s", bufs=2, name="tpq")
                for mt in range(MT):
                    nc.tensor.transpose(tp[:, mt * 128:(mt + 1) * 128],
                                        q_sb[g][mt][:, hc, :], ident[:])
                nc.vector.tensor_copy(qkT_sb[:, 0:SEG], tp[:])
                W()
                tp2 = psB.tile([D, SEG], bf16, tag="s", bufs=2, name="tpk")
                for mt in range(MT):
                    nc.tensor.transpose(tp2[:, mt * 128:(mt + 1) * 128],
                                        k_sb[g][mt][:, hc, :], ident[:])
                nc.vector.tensor_copy(qkT_sb[:, SEG:2 * SEG], tp2[:])
                W()

                av_ps = [psB.tile([128, MT // 2, D + 1], f32, tag=f"av{i}",
                                  bufs=1, name=f"av{i}") for i in range(2)]
                p_tiles = [None] * MT

                def emit_qk(kc):
                    s_ps = psB.tile([128, SEG], f32, tag="s", bufs=s_bufs,
                                    name="s")
                    for nn in range(2):
                        nc.tensor.matmul(
                            s_ps[:, nn * 512:(nn + 1) * 512],
                            qkT_sb[:, SEG + kc * 128:SEG + (kc + 1) * 128],
                            qkT_sb[:, nn * 512:(nn + 1) * 512],
                            start=True, stop=True)
                    p_sb = sbB.tile([128, SEG], bf16, tag="p", bufs=5,
                                    name="p")
                    nc.scalar.activation(p_sb[:], s_ps[:], Exp, scale=SCALE)
                    p_tiles[kc] = p_sb

                def emit_av(kc, g=g, hc=hc):
                    for qt in range(MT):
                        half, qi = divmod(qt, MT // 2)
                        nc.tensor.matmul(
                            av_ps[half][:, qi, :],
                            p_tiles[kc][:, qt * 128:(qt + 1) * 128],
                            v_sb[g][kc][:, hc, :],
                            start=(kc == 0 and qi == 0),
                            stop=(kc == MT - 1 and qi == MT // 2 - 1))

                def emit_avcopy(av_ps=av_ps):
                    # evict psum->SBUF right away so av_ps recycles for the
                    # next head without waiting on the rest of the tail
                    ao = sbB.tile([128, MT, D + 1], f32, tag="ao", bufs=2,
                                  name="ao")
                    for i in range(2):
                        nc.vector.tensor_copy(
                            ao[:, i * (MT // 2):(i + 1) * (MT // 2), :],
                            av_ps[i][:])
                    return ao

                def emit_tail(ao, h=h):
                    an = sbB.tile([128, MT, D], bf16, tag="an", name="an")
                    rcp = sbB.tile([128, MT], f32, tag="rcp", name="rcp")
                    nc.vector.reciprocal(rcp[:], ao[:, :, D])
                    # all-SBUF multiply: runs on Pool (DVE is the busy one)
                    nc.gpsimd.tensor_mul(
                        an[:], ao[:, :, 0:D],
                        rcp[:].unsqueeze(2).broadcast_to([128, MT, D]))
                    aT_ps = psB.tile([D, SEG], bf16, tag="pan", bufs=2,
                                     name="aTps")
                    for qt in range(MT):
                        nc.tensor.transpose(aT_ps[:, qt * 128:(qt + 1) * 128],
                                            an[:, qt, :], ident[:])
                    hi8 = sbB.tile([D, SEG], f8, tag="hi8", name="hi8")
                    nc.vector.tensor_copy(hi8[:], aT_ps[:])
                    lo8 = sbB.tile([D, SEG], f8, tag="lo8", name="lo8")
                    nc.vector.tensor_sub(lo8[:], aT_ps[:], hi8[:])
                    r0 = h * D
                    j = 0
                    while j < D:
                        r = r0 + j
                        k2, sl, p = r // 256, (r // 128) % 2, r % 128
                        ln = min(D - j, 128 - p)
                        nc.sync.dma_start(aT_hi[k2][p:p + ln, sl, :],
                                          hi8[j:j + ln, :])
                        nc.sync.dma_start(aT_lo[k2][p:p + ln, sl, :],
                                          lo8[j:j + ln, :])
                        j += ln

                emit_qk(0)
                emit_qk(1)
                W()
                if state["backlog"] is not None:
                    state["backlog"](6)
                emit_qk(2)
                if state["backlog"] is not None:
                    state["backlog"](7)
                    state["backlog"] = None
                if state["avcopy"] is not None:
                    state["tail_ao"] = state["avcopy"]()
                    state["avcopy"] = None
                W()
                emit_av(0)
                for kc in range(3, MT):
                    emit_qk(kc)
                    W()
                    emit_av(kc - 2)
                    if kc == 5 and state["tail"] is not None:
                        # deferred tail of the previous head: placed here so
                        # its PE transposes never delay this head's QKs
                        state["tail"](state["tail_ao"])
                        state["tail"] = None
                state["backlog"] = emit_av
                state["avcopy"] = emit_avcopy
                state["tail"] = emit_tail

            def drain():
                if state["backlog"] is not None:
                    state["backlog"](6)
                    state["backlog"](7)
                    state["backlog"] = None
                if state["avcopy"] is not None:
                    state["tail_ao"] = state["avcopy"]()
                    state["avcopy"] = None
                if state["tail"] is not None:
                    state["tail"](state["tail_ao"])
                    state["tail"] = None

            return emit_head, drain

        # ----- Scope 1: head-group 0 QKV (wide, ACT idle) -----------------
        with ExitStack() as s1:
            wp1 = s1.enter_context(tc.tile_pool(name="wp1", bufs=10))
            rtp1 = s1.enter_context(tc.tile_pool(name="rtp1", bufs=2))
            psA1 = s1.enter_context(tc.tile_pool(name="psA1", bufs=1,
                                                 space="PSUM"))
            for grp in range(3):
                chunk_wide(0, grp, wp1, rtp1, psA1)

        # ----- Scope 2: all heads, narrow QKV groups 1-2 woven in ---------
        with ExitStack() as s2:
            wp2 = s2.enter_context(tc.tile_pool(name="wp2", bufs=10))
            rtp2 = s2.enter_context(tc.tile_pool(name="rtp2", bufs=2))
            sbB1 = s2.enter_context(tc.tile_pool(name="sbB1", bufs=2))
            psB1 = s2.enter_context(tc.tile_pool(name="psB1", bufs=1,
                                                 space="PSUM"))

            progress = {"g": 1}

            def a2_rest():
                for g in (1, 2):
                    for grp in range(3):
                        yield from chunk_narrow_gen(g, grp, wp2, rtp2, psB1)
                    progress["g"] = g + 1
            gen = a2_rest()

            wrate = {"n": 2}

            def weave(gen=gen):
                for _ in range(wrate["n"]):
                    next(gen, None)

            emit_head, drain = make_head_emitter(sbB1, psB1, s_bufs=2)
            for h in range(H):
                # a head's q/k/v must be fully emitted before the head reads it
                need = 1 if h < 6 else (2 if h < 12 else 3)
                while progress["g"] < need and next(gen, "end") != "end":
                    pass
                wrate["n"] = 4 if h in (4, 5, 10, 11) else 2
                emit_head(h, weave=weave)
            drain()
            for _ in gen:
                pass

        # ----- Phase C: projection (3-term fp8 DR) ------------------------
        with ExitStack() as cctx:
            osb = cctx.enter_context(tc.tile_pool(name="osb", bufs=1))
            psC = cctx.enter_context(tc.tile_pool(name="psC", bufs=1,
                                                  space="PSUM"))
            NW = [(i * 256, 256) for i in range(5)]
            for mt in range(MT):
                pc = psC.tile([128, HID], f32, tag="pc", bufs=2, name="pc")
                for k2 in range(KK):
                    lh_hi = aT_hi[k2][:, :, mt * 128:(mt + 1) * 128]
                    lh_lo = aT_lo[k2][:, :, mt * 128:(mt + 1) * 128]
                    for (n0, nw) in NW:
                        dst = pc[:, n0:n0 + nw]
                        first = (k2 == 0) and (n0 % 512 == 0)
                        last = (k2 == KK - 1) and n0 in (256, 768, 1024)
                        nc.tensor.matmul(dst, lh_hi,
                                         wp_hi[k2][:, :, n0:n0 + nw],
                                         start=first, stop=False, perf_mode=DR)
                        nc.tensor.matmul(dst, lh_hi,
                                         wp_lo[k2][:, :, n0:n0 + nw],
                                         start=False, stop=False, perf_mode=DR)
                        nc.tensor.matmul(dst, lh_lo,
                                         wp_hi[k2][:, :, n0:n0 + nw],
                                         start=False, stop=last, perf_mode=DR)
                ot = osb.tile([128, HID], f32, tag="ot", bufs=2, name="ot")
                nc.scalar.activation(ot[:, 0:640], pc[:, 0:640], Copy,
                                     scale=1.0 / (OS * PS))
                nc.vector.tensor_scalar_mul(ot[:, 640:HID], pc[:, 640:HID],
                                            1.0 / (OS * PS))
                nc.sync.dma_start(out_dram[mt * 128:(mt + 1) * 128, :], ot[:])

    nc.compile()
    return nc


# ----- host-side prep (shared with v2) --------------------------------------

def _f8(x):
    import ml_dtypes
    return np.clip(x, -440.0, 440.0).astype(ml_dtypes.float8_e4m3)


def _hi_lo(x, scale):
    xs = np.asarray(x, np.float32) * scale
    hi = _f8(xs)
    lo = _f8(xs - hi.astype(np.float32))
    return hi, lo


def _col_perm():
    perm = []
    for (h0, h1) in HGROUPS:
        for grp in range(3):
            for h in range(h0, h1):
                base = grp * H * D + h * D
                perm.extend(range(base, base + D))
    return np.asarray(perm)


def _pack_pairs(a):
    n = a.shape[1]
    return np.ascontiguousarray(a.reshape(KK, 2, 128, n).transpose(0, 2, 1, 3))


def prepare_in_maps(hidden_states, cos, sin, wqkv, wproj):
    import ml_dtypes

    wq = wqkv[:, _col_perm()]
    w_hi, w_lo = _hi_lo(wq, WS)
    w_hi = _pack_pairs(w_hi).reshape(KK, 128, 2, 3 * HID)
    w_lo = _pack_pairs(w_lo).reshape(KK, 128, 2, 3 * HID)
    wp_hi, wp_lo = _hi_lo(wproj, PS)
    wp_hi = _pack_pairs(wp_hi).reshape(KK, 128, 2 * HID)
    wp_lo = _pack_pairs(wp_lo).reshape(KK, 128, 2 * HID)
    cos40 = np.ascontiguousarray(cos[:, 0:40]) / np.float32(HS * WS)
    sin40 = np.ascontiguousarray(sin[:, 0:40]) / np.float32(HS * WS)
    identbf = np.eye(128, dtype=ml_dtypes.bfloat16)

    in_maps = []
    for c in range(NSEG):
        rows = slice(c * SEG, (c + 1) * SEG)
        hT_hi, hT_lo = _hi_lo(hidden_states[rows].T, HS)
        in_maps.append({
            "hidT_hi": _pack_pairs(hT_hi).reshape(KK, 128, 2 * SEG),
            "hidT_lo": _pack_pairs(hT_lo).reshape(KK, 128, 2 * SEG),
            "w_hi": w_hi, "w_lo": w_lo,
            "wp_hi": wp_hi, "wp_lo": wp_lo,
            "cos40": np.ascontiguousarray(cos40[rows]),
            "sin40": np.ascontiguousarray(sin40[rows]),
            "identbf": identbf,
        })
    return in_maps


def _get_module():
    if "nc" not in _CACHE:
        _CACHE["nc"] = build_module(num_devices=NSEG)
    return _CACHE["nc"]


def kernel(hidden_states, cos, sin, qkv_kernel, qkv_bias, proj_kernel,
           proj_bias, cu_seqlens):
    from concourse import bass_utils

    hidden_states = np.ascontiguousarray(hidden_states, dtype=np.float32)
    cos = np.ascontiguousarray(cos, dtype=np.float32)
    sin = np.ascontiguousarray(sin, dtype=np.float32)
    wqkv = np.asarray(qkv_kernel, dtype=np.float32).reshape(HID, 3 * H * D)
    wproj = np.ascontiguousarray(proj_kernel, dtype=np.float32)

    assert not np.any(np.asarray(qkv_bias)), "nonzero qkv_bias unsupported"
    assert not np.any(np.asarray(proj_bias)), "nonzero proj_bias unsupported"
    expected_cu = np.arange(NSEG + 1, dtype=np.int64) * SEG
    assert np.array_equal(np.asarray(cu_seqlens, dtype=np.int64), expected_cu), \
        "kernel specialized for equal 1024-token segments"

    in_maps = prepare_in_maps(hidden_states, cos, sin, wqkv, wproj)
    nc = _get_module()
    res = bass_utils.run_bass_kernel_spmd(nc, in_maps,
                                          core_ids=list(range(NSEG)))
    out = np.concatenate([res.results[c]["out"] for c in range(NSEG)], axis=0)
    return out.astype(np.float32)


# revision 5
# speedup vs baseline: 1.0172x; 1.0056x over previous
"""Trainium2 Bass kernel for ExactVisionAttention — v3 (A2/B overlap).

Same math as v2 (3-term fp8e4 DoubleRow QKV/proj, bf16 QK/AV-swapped
attention, exp on ACT), restructured so the ACT exp stream starts ~70us
earlier: QKV head-group 0 computes first (wide, 8 psum banks), then
head-groups 1-2 run in a narrow 2-bank variant whose matmul groups are woven
between the first six heads' attention emission (B runs with a single s_ps
buffer during the overlap window), then heads 6-15 run at full rate.
"""

import os
import sys

for _p in ("/root/.axon_site/_ro/trn_rl_repo", "/opt/trn_rl_repo", "/root/.axon_site",
           "/root/.axon_site/_ro/pypackages"):
    if os.path.isdir(_p) and _p not in sys.path:
        sys.path.append(_p)

import numpy as np

S = 8192
HID = 1280
H = 16
D = 80
NSEG = 8
SEG = S // NSEG
MT = SEG // 128
KK = HID // 256
SCALE = float(D) ** -0.5

HS = 16.0
WS = 512.0
OS = 64.0
PS = 512.0

HGROUPS = [(0, 6), (6, 12), (12, 16)]

_CACHE = {}


def build_module(num_devices=8):
    import concourse.tile as tile
    from concourse import bacc, mybir
    from contextlib import ExitStack

    f32 = mybir.dt.float32
    bf16 = mybir.dt.bfloat16
    f8 = mybir.dt.float8e4
    Exp = mybir.ActivationFunctionType.Exp
    Copy = mybir.ActivationFunctionType.Copy
    DR = mybir.MatmulPerfMode.DoubleRow

    nc = bacc.Bacc("TRN2", target_bir_lowering=False, debug=False,
                   num_devices=num_devices)

    hidT_hi_in = nc.dram_tensor("hidT_hi", [KK, 128, 2 * SEG], f8,
                                kind="ExternalInput").ap()
    hidT_lo_in = nc.dram_tensor("hidT_lo", [KK, 128, 2 * SEG], f8,
                                kind="ExternalInput").ap()
    w_hi_in = nc.dram_tensor("w_hi", [KK, 128, 2, 3 * HID], f8,
                             kind="ExternalInput").ap()
    w_lo_in = nc.dram_tensor("w_lo", [KK, 128, 2, 3 * HID], f8,
                             kind="ExternalInput").ap()
    wp_hi_in = nc.dram_tensor("wp_hi", [KK, 128, 2 * HID], f8,
                              kind="ExternalInput").ap()
    wp_lo_in = nc.dram_tensor("wp_lo", [KK, 128, 2 * HID], f8,
                              kind="ExternalInput").ap()
    cos_in = nc.dram_tensor("cos40", [SEG, 40], f32, kind="ExternalInput").ap()
    sin_in = nc.dram_tensor("sin40", [SEG, 40], f32, kind="ExternalInput").ap()
    ident_in = nc.dram_tensor("identbf", [128, 128], bf16,
                              kind="ExternalInput").ap()
    out_dram = nc.dram_tensor("out", [SEG, HID], f32, kind="ExternalOutput").ap()

    with tile.TileContext(nc) as tc:
      with ExitStack() as ctx:
        constp = ctx.enter_context(tc.tile_pool(name="const", bufs=1))
        qkvp = ctx.enter_context(tc.tile_pool(name="qkvp", bufs=1))
        atp = ctx.enter_context(tc.tile_pool(name="atp", bufs=1))

        from concourse import library_config
        nc.gpsimd.load_library(library_config.proxy)

        # hidT first: the first QKV matmuls block on these (hi on SP, lo on
        # ACT so the two DGE queues split the transfer); weight DMAs follow
        # immediately on SP inside phase A.
        hidT_hi = [constp.tile([128, 2, SEG], f8, tag=f"hh{k2}", name=f"hh{k2}")
                   for k2 in range(KK)]
        hidT_lo = [constp.tile([128, 2, SEG], f8, tag=f"hl{k2}", name=f"hl{k2}")
                   for k2 in range(KK)]
        for k2 in range(KK):
            nc.sync.dma_start(hidT_hi[k2][:], hidT_hi_in[k2].rearrange(
                "p (two t) -> p two t", two=2))
            nc.scalar.dma_start(hidT_lo[k2][:], hidT_lo_in[k2].rearrange(
                "p (two t) -> p two t", two=2))

        cos40 = [constp.tile([128, 40], f32, tag=f"c{mt}", name=f"c{mt}")
                 for mt in range(MT)]
        sin40 = [constp.tile([128, 40], f32, tag=f"s{mt}", name=f"s{mt}")
                 for mt in range(MT)]
        # SWDGE queue: keeps the ACT hwdge queue clear so the wide-phase
        # stage copies (which free the QKV psum banks) are not stuck behind
        # DMA issue slots
        for mt in range(MT):
            nc.gpsimd.dma_start(cos40[mt][:], cos_in[mt * 128:(mt + 1) * 128, :])
            nc.gpsimd.dma_start(sin40[mt][:], sin_in[mt * 128:(mt + 1) * 128, :])
        ident = constp.tile([128, 128], bf16, tag="ident", name="ident")
        nc.gpsimd.dma_start(ident[:], ident_in[:])

        # q/k/v split per head-group so overlapped A2 writes (groups 1-2)
        # never false-depend against B reads of group-0 heads
        NHG = [hg[1] - hg[0] for hg in HGROUPS]
        q_sb = [[qkvp.tile([128, NHG[g], D], bf16, tag=f"q{g}_{mt}",
                           name=f"q{g}_{mt}") for mt in range(MT)]
                for g in range(3)]
        k_sb = [[qkvp.tile([128, NHG[g], D], bf16, tag=f"k{g}_{mt}",
                           name=f"k{g}_{mt}") for mt in range(MT)]
                for g in range(3)]
        v_sb = [[qkvp.tile([128, NHG[g], D + 1], bf16, tag=f"v{g}_{mt}",
                           name=f"v{g}_{mt}") for mt in range(MT)]
                for g in range(3)]
        for g in range(3):
            for mt in range(MT):
                nc.gpsimd.memset(v_sb[g][mt][:, :, D:D + 1], 1.0 / OS)

        aT_hi = [atp.tile([128, 2, SEG], f8, tag=f"ah{k2}", name=f"ah{k2}")
                 for k2 in range(KK)]
        aT_lo = [atp.tile([128, 2, SEG], f8, tag=f"al{k2}", name=f"al{k2}")
                 for k2 in range(KK)]
        wp_hi = [atp.tile([128, 2, HID], f8, tag=f"wph{k2}", name=f"wph{k2}")
                 for k2 in range(KK)]
        wp_lo = [atp.tile([128, 2, HID], f8, tag=f"wpl{k2}", name=f"wpl{k2}")
                 for k2 in range(KK)]
        for k2 in range(KK):
            nc.gpsimd.dma_start(wp_hi[k2][:], wp_hi_in[k2].rearrange(
                "p (two e) -> p two e", two=2))
            nc.gpsimd.dma_start(wp_lo[k2][:], wp_lo_in[k2].rearrange(
                "p (two e) -> p two e", two=2))

        # ----- shared emitters -------------------------------------------

        def rope_evict(g, grp, mt, ps, nh, h0g, rtp, stage_on_act):
            """ps: [128, nh*D] psum (or staged sbuf) view for this mt."""
            if grp == 2:
                nc.vector.tensor_scalar_mul(
                    v_sb[g][mt][:, :, 0:D],
                    ps.rearrange("p (h d) -> p h d", h=nh), 1.0 / (HS * WS))
                return
            if stage_on_act:
                qs = rtp.tile([128, 512], f32, tag="qs", bufs=4, name="qs")
                nc.scalar.copy(qs[:, 0:nh * D], ps)
                ps = qs[:, 0:nh * D]
            dst = q_sb[g][mt] if grp == 0 else k_sb[g][mt]
            ps3 = ps.rearrange("p (h d) -> p h d", h=nh)
            ps4 = ps.rearrange("p (h two d) -> p h two d", h=nh, two=2)
            cos_bc4 = (cos40[mt][:].unsqueeze(1).unsqueeze(2)
                       .broadcast_to([128, nh, 2, 40]))
            sin_bc3 = (sin40[mt][:].unsqueeze(1).broadcast_to([128, nh, 40]))
            t = rtp.tile([128, 6, D], f32, tag="t", name="t")
            t4 = t[:, 0:nh, :].rearrange("p h (two d) -> p h two d", two=2)
            nc.vector.tensor_mul(t4, ps4, cos_bc4)
            m1 = rtp.tile([128, 6, 40], f32, tag="m1", name="m1")
            nc.vector.tensor_mul(m1[:, 0:nh, :], ps3[:, :, 40:80], sin_bc3)
            m2 = rtp.tile([128, 6, 40], f32, tag="m2", name="m2")
            nc.vector.tensor_mul(m2[:, 0:nh, :], ps3[:, :, 0:40], sin_bc3)
            nc.gpsimd.tensor_sub(dst[:, :, 0:40], t[:, 0:nh, 0:40],
                                 m1[:, 0:nh, :])
            nc.gpsimd.tensor_add(dst[:, :, 40:80], m2[:, 0:nh, :],
                                 t[:, 0:nh, 40:80])

        def chunk_wide(g, grp, wp, rtp, psA):
            """8-bank QKV chunk: all 8 mt tiles accumulate concurrently."""
            h0, h1 = HGROUPS[g]
            nh = h1 - h0
            cw = nh * D
            c0 = (h0 * 3 + grp * nh) * D
            pss = [psA.tile([128, 512], f32, tag=f"pa{mt}", name=f"pa{mt}")
                   for mt in range(MT)]
            nn_splits = [(0, cw // 2), (cw // 2, cw // 2)]
            for k2 in range(KK):
                wt_hi = wp.tile([128, 2, 512], f8, tag="wth", name="wth",
                                bufs=10)
                nc.sync.dma_start(wt_hi[:, :, 0:cw],
                                  w_hi_in[k2, :, :, c0:c0 + cw])
                wt_lo = wp.tile([128, 2, 512], f8, tag="wtl", name="wtl",
                                bufs=10)
                nc.sync.dma_start(wt_lo[:, :, 0:cw],
                                  w_lo_in[k2, :, :, c0:c0 + cw])
                for mt in range(MT):
                    lhs_hi = hidT_hi[k2][:, :, mt * 128:(mt + 1) * 128]
                    lhs_lo = hidT_lo[k2][:, :, mt * 128:(mt + 1) * 128]
                    for si, (n0, nw) in enumerate(nn_splits):
                        dst = pss[mt][:, n0:n0 + nw]
                        first = (k2 == 0) and (si == 0)
                        last = (k2 == KK - 1) and (si == len(nn_splits) - 1)
                        nc.tensor.matmul(dst, lhs_hi, wt_hi[:, :, n0:n0 + nw],
                                         start=first, stop=False, perf_mode=DR)
                        nc.tensor.matmul(dst, lhs_hi, wt_lo[:, :, n0:n0 + nw],
                                         start=False, stop=False, perf_mode=DR)
                        nc.tensor.matmul(dst, lhs_lo, wt_hi[:, :, n0:n0 + nw],
                                         start=False, stop=last, perf_mode=DR)
            for mt in range(MT):
                rope_evict(g, grp, mt, pss[mt][:, 0:cw], nh, h0, rtp,
                           stage_on_act=True)

        def chunk_narrow_gen(g, grp, wp, rtp, psA):
            """2-bank QKV chunk as a generator: yields after each (pass, k2)
            matmul group so B-head emission can weave between them."""
            h0, h1 = HGROUPS[g]
            nh = h1 - h0
            cw = nh * D
            c0 = (h0 * 3 + grp * nh) * D
            wts = []
            for k2 in range(KK):
                wt_hi = wp.tile([128, 2, 512], f8, tag="wth", name="wth",
                                bufs=10)
                nc.sync.dma_start(wt_hi[:, :, 0:cw],
                                  w_hi_in[k2, :, :, c0:c0 + cw])
                wt_lo = wp.tile([128, 2, 512], f8, tag="wtl", name="wtl",
                                bufs=10)
                nc.sync.dma_start(wt_lo[:, :, 0:cw],
                                  w_lo_in[k2, :, :, c0:c0 + cw])
                wts.append((wt_hi, wt_lo))
            nn_splits = [(0, cw // 2), (cw // 2, cw // 2)]
            for mt in range(MT):
                ps = psA.tile([128, 512], f32, tag="pan", name="pan", bufs=2)
                for k2 in range(KK):
                    wt_hi, wt_lo = wts[k2]
                    lhs_hi = hidT_hi[k2][:, :, mt * 128:(mt + 1) * 128]
                    lhs_lo = hidT_lo[k2][:, :, mt * 128:(mt + 1) * 128]
                    for si, (n0, nw) in enumerate(nn_splits):
                        dst = ps[:, n0:n0 + nw]
                        first = (k2 == 0) and (si == 0)
                        last = (k2 == KK - 1) and (si == 1)
                        nc.tensor.matmul(dst, lhs_hi, wt_hi[:, :, n0:n0 + nw],
                                         start=first, stop=False, perf_mode=DR)
                        nc.tensor.matmul(dst, lhs_hi, wt_lo[:, :, n0:n0 + nw],
                                         start=False, stop=False, perf_mode=DR)
                        nc.tensor.matmul(dst, lhs_lo, wt_hi[:, :, n0:n0 + nw],
                                         start=False, stop=last, perf_mode=DR)
                    yield
                # RoPE reads psum directly (ACT stays exp-only here)
                rope_evict(g, grp, mt, ps[:, 0:cw], nh, h0, rtp,
                           stage_on_act=False)
                yield

        def make_head_emitter(sbB, psB, s_bufs):
            state = {"backlog": None, "avcopy": None, "tail": None,
                     "tail_ao": None}

            def qkv_of(h):
                g = 0 if h < 6 else (1 if h < 12 else 2)
                return g, h - HGROUPS[g][0]

            def emit_head(h, weave=None):
                def W():
                    if weave is not None:
                        weave()
                g, hc = qkv_of(h)
                # late heads: the A2 generator is exhausted (head-12 barrier),
                # so its 2 pan banks are free - transposes there skip the
                # exp-slot wait and the boundary gap shrinks
                ttag = "pan" if h >= 12 else "s"
                qkT_sb = sbB.tile([D, 2 * SEG], bf16, tag="qkT", bufs=2,
                                  name="qkT")
                tp = psB.tile([D, SEG], bf16, tag=ttag, bufs=2, name="tpq")
                for mt in range(MT):
                    nc.tensor.transpose(tp[:, mt * 128:(mt + 1) * 128],
                                        q_sb[g][mt][:, hc, :], ident[:])
                nc.vector.tensor_copy(qkT_sb[:, 0:SEG], tp[:])
                W()
                tp2 = psB.tile([D, SEG], bf16, tag=ttag, bufs=2, name="tpk")
                for mt in range(MT):
                    nc.tensor.transpose(tp2[:, mt * 128:(mt + 1) * 128],
                                        k_sb[g][mt][:, hc, :], ident[:])
                nc.vector.tensor_copy(qkT_sb[:, SEG:2 * SEG], tp2[:])
                W()

                av_ps = [psB.tile([128, MT // 2, D + 1], f32, tag=f"av{i}",
                                  bufs=1, name=f"av{i}") for i in range(2)]
                p_tiles = [None] * MT

                def emit_qk(kc):
                    s_ps = psB.tile([128, SEG], f32, tag="s", bufs=s_bufs,
                                    name="s")
                    for nn in range(2):
                        nc.tensor.matmul(
                            s_ps[:, nn * 512:(nn + 1) * 512],
                            qkT_sb[:, SEG + kc * 128:SEG + (kc + 1) * 128],
                            qkT_sb[:, nn * 512:(nn + 1) * 512],
                            start=True, stop=True)
                    p_sb = sbB.tile([128, SEG], bf16, tag="p", bufs=5,
                                    name="p")
                    nc.scalar.activation(p_sb[:], s_ps[:], Exp, scale=SCALE)
                    p_tiles[kc] = p_sb

                def emit_av(kc, g=g, hc=hc):
                    for qt in range(MT):
                        half, qi = divmod(qt, MT // 2)
                        nc.tensor.matmul(
                            av_ps[half][:, qi, :],
                            p_tiles[kc][:, qt * 128:(qt + 1) * 128],
                            v_sb[g][kc][:, hc, :],
                            start=(kc == 0 and qi == 0),
                            stop=(kc == MT - 1 and qi == MT // 2 - 1))

                def emit_avcopy(av_ps=av_ps):
                    # evict psum->SBUF right away so av_ps recycles for the
                    # next head without waiting on the rest of the tail
                    ao = sbB.tile([128, MT, D + 1], f32, tag="ao", bufs=2,
                                  name="ao")
                    for i in range(2):
                        nc.vector.tensor_copy(
                            ao[:, i * (MT // 2):(i + 1) * (MT // 2), :],
                            av_ps[i][:])
                    return ao

                def emit_tail(ao, h=h):
                    an = sbB.tile([128, MT, D], bf16, tag="an", name="an")
                    rcp = sbB.tile([128, MT], f32, tag="rcp", name="rcp")
                    nc.vector.reciprocal(rcp[:], ao[:, :, D])
                    # all-SBUF multiply: runs on Pool (DVE is the busy one)
                    nc.gpsimd.tensor_mul(
                        an[:], ao[:, :, 0:D],
                        rcp[:].unsqueeze(2).broadcast_to([128, MT, D]))
                    aT_ps = psB.tile([D, SEG], bf16, tag="pan", bufs=2,
                                     name="aTps")
                    for qt in range(MT):
                        nc.tensor.transpose(aT_ps[:, qt * 128:(qt + 1) * 128],
                                            an[:, qt, :], ident[:])
                    hi8 = sbB.tile([D, SEG], f8, tag="hi8", name="hi8")
                    nc.vector.tensor_copy(hi8[:], aT_ps[:])
                    lo8 = sbB.tile([D, SEG], f8, tag="lo8", name="lo8")
                    nc.vector.tensor_sub(lo8[:], aT_ps[:], hi8[:])
                    r0 = h * D
                    j = 0
                    while j < D:
                        r = r0 + j
                        k2, sl, p = r // 256, (r // 128) % 2, r % 128
                        ln = min(D - j, 128 - p)
                        nc.sync.dma_start(aT_hi[k2][p:p + ln, sl, :],
                                          hi8[j:j + ln, :])
                        nc.sync.dma_start(aT_lo[k2][p:p + ln, sl, :],
                                          lo8[j:j + ln, :])
                        j += ln

                emit_qk(0)
                emit_qk(1)
                W()
                if state["backlog"] is not None:
                    state["backlog"](6)
                emit_qk(2)
                if state["backlog"] is not None:
                    state["backlog"](7)
                    state["backlog"] = None
                if state["avcopy"] is not None:
                    state["tail_ao"] = state["avcopy"]()
                    state["avcopy"] = None
                W()
                emit_av(0)
                for kc in range(3, MT):
                    emit_qk(kc)
                    W()
                    emit_av(kc - 2)
                    if kc == 5 and state["tail"] is not None:
                        # deferred tail of the previous head: placed here so
                        # its PE transposes never delay this head's QKs
                        state["tail"](state["tail_ao"])
                        state["tail"] = None
                state["backlog"] = emit_av
                state["avcopy"] = emit_avcopy
                state["tail"] = emit_tail

            def drain():
                if state["backlog"] is not None:
                    state["backlog"](6)
                    state["backlog"](7)
                    state["backlog"] = None
                if state["avcopy"] is not None:
                    state["tail_ao"] = state["avcopy"]()
                    state["avcopy"] = None
                if state["tail"] is not None:
                    state["tail"](state["tail_ao"])
                    state["tail"] = None

            return emit_head, drain

        # ----- Scope 1: head-group 0 QKV (wide, ACT idle) -----------------
        with ExitStack() as s1:
            wp1 = s1.enter_context(tc.tile_pool(name="wp1", bufs=10))
            rtp1 = s1.enter_context(tc.tile_pool(name="rtp1", bufs=2))
            psA1 = s1.enter_context(tc.tile_pool(name="psA1", bufs=1,
                                                 space="PSUM"))
            for grp in range(3):
                chunk_wide(0, grp, wp1, rtp1, psA1)

        # ----- Scope 2: all heads, narrow QKV groups 1-2 woven in ---------
        with ExitStack() as s2:
            wp2 = s2.enter_context(tc.tile_pool(name="wp2", bufs=10))
            rtp2 = s2.enter_context(tc.tile_pool(name="rtp2", bufs=2))
            sbB1 = s2.enter_context(tc.tile_pool(name="sbB1", bufs=2))
            psB1 = s2.enter_context(tc.tile_pool(name="psB1", bufs=1,
                                                 space="PSUM"))

            progress = {"g": 1}

            def a2_rest():
                for g in (1, 2):
                    for grp in range(3):
                        yield from chunk_narrow_gen(g, grp, wp2, rtp2, psB1)
                    progress["g"] = g + 1
            gen = a2_rest()

            wrate = {"n": 2}

            def weave(gen=gen):
                for _ in range(wrate["n"]):
                    next(gen, None)

            emit_head, drain = make_head_emitter(sbB1, psB1, s_bufs=2)
            for h in range(H):
                # a head's q/k/v must be fully emitted before the head reads it
                need = 1 if h < 6 else (2 if h < 12 else 3)
                while progress["g"] < need and next(gen, "end") != "end":
                    pass
                wrate["n"] = 4 if h in (4, 5, 10, 11) else 2
                emit_head(h, weave=weave)
            drain()
            for _ in gen:
                pass

        # ----- Phase C: projection (3-term fp8 DR) ------------------------
        with ExitStack() as cctx:
            osb = cctx.enter_context(tc.tile_pool(name="osb", bufs=1))
            psC = cctx.enter_context(tc.tile_pool(name="psC", bufs=1,
                                                  space="PSUM"))
            NW = [(i * 256, 256) for i in range(5)]
            for mt in range(MT):
                pc = psC.tile([128, HID], f32, tag="pc", bufs=2, name="pc")
                for k2 in range(KK):
                    lh_hi = aT_hi[k2][:, :, mt * 128:(mt + 1) * 128]
                    lh_lo = aT_lo[k2][:, :, mt * 128:(mt + 1) * 128]
                    for (n0, nw) in NW:
                        dst = pc[:, n0:n0 + nw]
                        first = (k2 == 0) and (n0 % 512 == 0)
                        last = (k2 == KK - 1) and n0 in (256, 768, 1024)
                        nc.tensor.matmul(dst, lh_hi,
                                         wp_hi[k2][:, :, n0:n0 + nw],
                                         start=first, stop=False, perf_mode=DR)
                        nc.tensor.matmul(dst, lh_hi,
                                         wp_lo[k2][:, :, n0:n0 + nw],
                                         start=False, stop=False, perf_mode=DR)
                        nc.tensor.matmul(dst, lh_lo,
                                         wp_hi[k2][:, :, n0:n0 + nw],
                                         start=False, stop=last, perf_mode=DR)
                ot = osb.tile([128, HID], f32, tag="ot", bufs=2, name="ot")
                nc.scalar.activation(ot[:, 0:640], pc[:, 0:640], Copy,
                                     scale=1.0 / (OS * PS))
                nc.vector.tensor_scalar_mul(ot[:, 640:HID], pc[:, 640:HID],
                                            1.0 / (OS * PS))
                nc.sync.dma_start(out_dram[mt * 128:(mt + 1) * 128, :], ot[:])

    nc.compile()
    return nc


# ----- host-side prep (shared with v2) --------------------------------------

def _f8(x):
    import ml_dtypes
    return np.clip(x, -440.0, 440.0).astype(ml_dtypes.float8_e4m3)


def _hi_lo(x, scale):
    xs = np.asarray(x, np.float32) * scale
    hi = _f8(xs)
    lo = _f8(xs - hi.astype(np.float32))
    return hi, lo


def _col_perm():
    perm = []
    for (h0, h1) in HGROUPS:
        for grp in range(3):
            for h in range(h0, h1):
                base = grp * H * D + h * D
                perm.extend(range(base, base + D))
    return np.asarray(perm)


def _pack_pairs(a):
    n = a.shape[1]
    return np.ascontiguousarray(a.reshape(KK, 2, 128, n).transpose(0, 2, 1, 3))


def prepare_in_maps(hidden_states, cos, sin, wqkv, wproj):
    import ml_dtypes

    wq = wqkv[:, _col_perm()]
    w_hi, w_lo = _hi_lo(wq, WS)
    w_hi = _pack_pairs(w_hi).reshape(KK, 128, 2, 3 * HID)
    w_lo = _pack_pairs(w_lo).reshape(KK, 128, 2, 3 * HID)
    wp_hi, wp_lo = _hi_lo(wproj, PS)
    wp_hi = _pack_pairs(wp_hi).reshape(KK, 128, 2 * HID)
    wp_lo = _pack_pairs(wp_lo).reshape(KK, 128, 2 * HID)
    cos40 = np.ascontiguousarray(cos[:, 0:40]) / np.float32(HS * WS)
    sin40 = np.ascontiguousarray(sin[:, 0:40]) / np.float32(HS * WS)
    identbf = np.eye(128, dtype=ml_dtypes.bfloat16)

    in_maps = []
    for c in range(NSEG):
        rows = slice(c * SEG, (c + 1) * SEG)
        hT_hi, hT_lo = _hi_lo(hidden_states[rows].T, HS)
        in_maps.append({
            "hidT_hi": _pack_pairs(hT_hi).reshape(KK, 128, 2 * SEG),
            "hidT_lo": _pack_pairs(hT_lo).reshape(KK, 128, 2 * SEG),
            "w_hi": w_hi, "w_lo": w_lo,
            "wp_hi": wp_hi, "wp_lo": wp_lo,
            "cos40": np.ascontiguousarray(cos40[rows]),
            "sin40": np.ascontiguousarray(sin40[rows]),
            "identbf": identbf,
        })
    return in_maps


def _get_module():
    if "nc" not in _CACHE:
        _CACHE["nc"] = build_module(num_devices=NSEG)
    return _CACHE["nc"]


def kernel(hidden_states, cos, sin, qkv_kernel, qkv_bias, proj_kernel,
           proj_bias, cu_seqlens):
    from concourse import bass_utils

    hidden_states = np.ascontiguousarray(hidden_states, dtype=np.float32)
    cos = np.ascontiguousarray(cos, dtype=np.float32)
    sin = np.ascontiguousarray(sin, dtype=np.float32)
    wqkv = np.asarray(qkv_kernel, dtype=np.float32).reshape(HID, 3 * H * D)
    wproj = np.ascontiguousarray(proj_kernel, dtype=np.float32)

    assert not np.any(np.asarray(qkv_bias)), "nonzero qkv_bias unsupported"
    assert not np.any(np.asarray(proj_bias)), "nonzero proj_bias unsupported"
    expected_cu = np.arange(NSEG + 1, dtype=np.int64) * SEG
    assert np.array_equal(np.asarray(cu_seqlens, dtype=np.int64), expected_cu), \
        "kernel specialized for equal 1024-token segments"

    in_maps = prepare_in_maps(hidden_states, cos, sin, wqkv, wproj)
    nc = _get_module()
    res = bass_utils.run_bass_kernel_spmd(nc, in_maps,
                                          core_ids=list(range(NSEG)))
    out = np.concatenate([res.results[c]["out"] for c in range(NSEG)], axis=0)
    return out.astype(np.float32)
